# revision 27
# baseline (speedup 1.0000x reference)
"""NeRF attention Bass kernel for 8 Trainium2 NeuronCores.

Math (from the reference):
    pos = rays.reshape(N, 16),  f = features.reshape(N, 64),  N = 8192
    Q = LN(relu(pos@Wq1+bq1)@Wq2+bq2)*gq+bq_ln / 8           [N, 64]
    K = LN(relu(pos@Wk1+bk1)@Wk2+bk2)*gk+bk_ln               [N, 64]
    attn = softmax((Q @ K.T) * norm_scale, axis=-1)          [N, N]
    attn_out = attn @ f                                      [N, 64]
    returns (attn_out.reshape(8,32,32,64), attn)

Sharding: rows of Q / attn / attn_out are split across 8 cores (1024 rows
each); K, features and the tiny MLP params are replicated.

Device-side structure (per core, all feature-major / transposed):
  1. posT [17, n] built via PE transposes of rays tiles (ones bias row).
  2. MLPs: LayerNorm's centering and gain are LINEAR, so they are folded
     into W2 host-side:  t0g = W2cg.T @ relu(W1a.T @ posT_aug) is already
     centered-and-scaled; var comes from one PE matmul with a host-built
     1/(64*g^2) broadcast matrix; then rstd (ACT sqrt + DVE reciprocal)
     and out = t0g * rstdB + b.  norm_scale/8 is folded into Q's coeffs.
  3. phase B (token-major rows): scores tile = QT_blk.T @ KT chunk (PE,
     f32r) -> exp on ACT (accum_out emits row-sums for free) -> one DVE
     tensor_scalar normalize -> one 4MB row-block DMA out.
  4. phase C (key-major): scoresT = KT_tile.T @ QT (PE) -> exp (ACT) ->
     PV matmul accumulating attn_out.T over 64 key tiles; PE-transpose
     back to token-major and scale by the phase-B reciprocals.

Softmax skips the max-subtraction: scores are bounded (|s| < ~6 for this
problem family) so exp never overflows; this matches jax to ~1e-5.
Matmuls read fp32 data as float32r (fp22) which runs the PE at full rate.
"""

from contextlib import ExitStack

import numpy as np

import concourse.bass as bass
import concourse.mybir as mybir
import concourse.tile as tile
from concourse import bacc
from concourse.bass_utils import run_bass_kernel_spmd

F32 = mybir.dt.float32
F32R = mybir.dt.float32r
AF = mybir.ActivationFunctionType
ALU = mybir.AluOpType
AX = mybir.AxisListType

N_CORES = 8
N = 8192          # total tokens (8*32*32)
NPOS = 16
NHID = 32
NE = 64           # embed dim == feature dim
SHARD = N // N_CORES          # 1024 rows per core
CHUNK = 512                   # free-dim chunk for matmuls / MLP
NCH = N // CHUNK              # 16 K-chunks
NCH_Q = SHARD // CHUNK        # 2 Q-chunks
NBLK = SHARD // 128           # 8 token blocks per core
NKT = N // 128                # 64 key tiles
EPS = 1e-5

_CACHE = {}


def _r(ap):
    """Read an fp32 AP as float32r (fp22 truncated, full-rate PE)."""
    return ap.bitcast(F32R)


def _build_program():
    nc = bacc.Bacc("TRN2", target_bir_lowering=False, debug=False)

    rays_all = nc.dram_tensor("rays_all", [N, NPOS], F32, kind="ExternalInput").ap()
    rays_sh = nc.dram_tensor("rays_sh", [SHARD, NPOS], F32, kind="ExternalInput").ap()
    feats = nc.dram_tensor("feats", [N, NE], F32R, kind="ExternalInput").ap()
    id128 = nc.dram_tensor("id128", [128, 128], F32, kind="ExternalInput").ap()
    # per-feature LN shift columns: bk_ln, bq_ln' (q pre-scaled by ns/8)
    lncoef = nc.dram_tensor("lncoef", [NE, 2], F32, kind="ExternalInput").ap()
    wk1a = nc.dram_tensor("wk1a", [NPOS + 1, NHID], F32R, kind="ExternalInput").ap()
    wk2c = nc.dram_tensor("wk2c", [NHID + 1, NE], F32R, kind="ExternalInput").ap()
    wq1a = nc.dram_tensor("wq1a", [NPOS + 1, NHID], F32R, kind="ExternalInput").ap()
    wq2c = nc.dram_tensor("wq2c", [NHID + 1, NE], F32R, kind="ExternalInput").ap()
    vark = nc.dram_tensor("vark", [NE, NE], F32R, kind="ExternalInput").ap()
    varq = nc.dram_tensor("varq", [NE, NE], F32R, kind="ExternalInput").ap()

    attn_w = nc.dram_tensor("attn_w", [SHARD, N], F32, kind="ExternalOutput").ap()
    attn_o = nc.dram_tensor("attn_o", [SHARD, NE], F32, kind="ExternalOutput").ap()

    with tile.TileContext(nc) as tc, ExitStack() as ctx:
        const = ctx.enter_context(tc.tile_pool(name="const", bufs=1))
        persist = ctx.enter_context(tc.tile_pool(name="persist", bufs=1))

        # ---------------- constants / inputs to SBUF ----------------
        ident = const.tile([128, 128], F32, tag="ident")
        nc.sync.dma_start(ident[:], id128)
        lnc = const.tile([NE, 2], F32, tag="lnc")
        nc.sync.dma_start(lnc[:], lncoef)
        w_k1 = const.tile([NPOS + 1, NHID], F32R, tag="wk1")
        nc.sync.dma_start(w_k1[:], wk1a)
        w_k2 = const.tile([NHID + 1, NE], F32R, tag="wk2")
        nc.sync.dma_start(w_k2[:], wk2c)
        w_q1 = const.tile([NPOS + 1, NHID], F32R, tag="wq1")
        nc.sync.dma_start(w_q1[:], wq1a)
        w_q2 = const.tile([NHID + 1, NE], F32R, tag="wq2")
        nc.sync.dma_start(w_q2[:], wq2c)
        v_k = const.tile([NE, NE], F32R, tag="vark")
        nc.sync.dma_start(v_k[:], vark)
        v_q = const.tile([NE, NE], F32R, tag="varq")
        nc.sync.dma_start(v_q[:], varq)
        eps_col = const.tile([128, 1], F32, tag="eps")
        nc.vector.memset(eps_col[:], EPS)

        # features, token-tiled: f_sb[:, kt*64:(kt+1)*64] = feats[kt*128:+128, :]
        f_sb = persist.tile([128, NKT * NE], F32R, tag="f")
        nc.sync.dma_start(
            f_sb[:].rearrange("p (t d) -> p t d", t=NKT),
            feats.rearrange("(t p) d -> p t d", p=128),
        )

        # ---------------- posT via PE transposes ----------------
        def build_posT(rays_ap, n_tok, tag, mlppool, psA):
            nt = n_tok // 128
            pos_sb = mlppool.tile([128, nt * NPOS], F32, tag=tag + "_tm")
            nc.sync.dma_start(
                pos_sb[:].rearrange("p (t j) -> p t j", t=nt),
                rays_ap.rearrange("(t p) j -> p t j", p=128),
            )
            posTa = mlppool.tile([NPOS + 1, n_tok], F32R, tag=tag)
            # bias row (NPOS) must be ones; memset whole tile, transposes
            # overwrite rows 0-15 (DVE needs 32-aligned start partitions)
            nc.vector.memset(posTa[:].bitcast(F32), 1.0)
            for g in range(0, nt, 4):
                tr = psA.tile([NPOS, 4 * 128], F32, tag="ptr")
                for i in range(4):
                    t = g + i
                    nc.tensor.transpose(
                        tr[:, i * 128 : (i + 1) * 128],
                        pos_sb[:, t * NPOS : (t + 1) * NPOS],
                        ident[:],
                    )
                nc.vector.tensor_copy(posTa[0:NPOS, g * 128 : (g + 4) * 128], tr[:])
            return posTa

        # ---------------- feature-major MLP -> transposed embeddings ----------
        def mlp(posTa, w1, w2c, vmat, b_col, nch, tag, mlpsb, psA, psB):
            """Returns nch tiles [NE, CHUNK]: the normalized embedding^T.

            w2c is the centered-and-gain-scaled second layer, so its matmul
            output t0g is (a - mean(a)) * g directly; vmat = 1/(64*g^2)
            broadcast matrix gives varB = var(a) replicated on every row.
            """
            out_tiles = []
            for c in range(nch):
                cols = slice(c * CHUNK, (c + 1) * CHUNK)
                h_ps = psA.tile([NHID, CHUNK], F32, tag="mlp_h")
                nc.tensor.matmul(h_ps[:], w1[:], _r(posTa[:, cols]))
                h_sb = mlpsb.tile([NHID + 1, CHUNK], F32R, tag="mlp_h")
                nc.vector.memset(h_sb[NHID : NHID + 1, :].bitcast(F32), 1.0)
                nc.vector.tensor_scalar(
                    h_sb[0:NHID, :], h_ps[:], 0.0, None, op0=ALU.max
                )
                t0_ps = psA.tile([NE, CHUNK], F32, tag="mlp_t0")
                nc.tensor.matmul(t0_ps[:], w2c[:], h_sb[:])
                t0_sb = mlpsb.tile([NE, CHUNK], F32R, tag="mlp_t0")
                nc.vector.tensor_copy(t0_sb[:], t0_ps[:])
                sqd = mlpsb.tile([NE, CHUNK], F32R, tag="mlp_sqd")
                nc.vector.tensor_tensor(
                    sqd[:], t0_sb[:].bitcast(F32), t0_sb[:].bitcast(F32), op=ALU.mult
                )
                varB = psB.tile([NE, CHUNK], F32, tag="varB")
                nc.tensor.matmul(varB[:], vmat[:], sqd[:])
                sqB = mlpsb.tile([NE, CHUNK], F32, tag="mlp_sqB")
                nc.scalar.activation(sqB[:], varB[:], AF.Sqrt, bias=eps_col[0:NE, :])
                rstdB = mlpsb.tile([NE, CHUNK], F32, tag="mlp_rstd")
                nc.vector.reciprocal_approx_fast(rstdB[:], sqB[:])
                t1 = mlpsb.tile([NE, CHUNK], F32, tag="mlp_t1")
                nc.vector.tensor_tensor(
                    t1[:], t0_sb[:].bitcast(F32), rstdB[:], op=ALU.mult
                )
                o = persist.tile([NE, CHUNK], F32R, tag=f"{tag}_o{c}")
                nc.vector.tensor_scalar(o[:], t1[:], b_col, None, op0=ALU.add)
                out_tiles.append(o)
            return out_tiles

        with (
            tc.tile_pool(name="mlppool", bufs=1) as mlppool,
            tc.tile_pool(name="mlpsb", bufs=3) as mlpsb,
            tc.tile_pool(name="psA", bufs=2, space="PSUM") as psA,
            tc.tile_pool(name="psB", bufs=2, space="PSUM") as psB,
        ):
            posT_k = build_posT(rays_all, N, "posk", mlppool, psA)
            posT_q = build_posT(rays_sh, SHARD, "posq", mlppool, psA)
            kt_tiles = mlp(posT_k, w_k1, w_k2, v_k, lnc[:, 0:1],
                           NCH, "k", mlpsb, psA, psB)
            qt_tiles = mlp(posT_q, w_q1, w_q2, v_q, lnc[:, 1:2],
                           NCH_Q, "q", mlpsb, psA, psB)

        exppool = ctx.enter_context(tc.tile_pool(name="exppool", bufs=2))
        ctpool = ctx.enter_context(tc.tile_pool(name="ctpool", bufs=4))
        smpool = ctx.enter_context(tc.tile_pool(name="smpool", bufs=3))
        psum = ctx.enter_context(tc.tile_pool(name="psum", bufs=1, space="PSUM"))
        psum2 = ctx.enter_context(tc.tile_pool(name="psum2", bufs=2, space="PSUM"))
        psum3 = ctx.enter_context(tc.tile_pool(name="psum3", bufs=3, space="PSUM"))

        recip_all = persist.tile([128, NBLK], F32, tag="recip")

        # ---------------- phase B: token-major attn rows ----------------
        def phase_b(b):
            qt = qt_tiles[b // 4][:, (b % 4) * 128 : (b % 4 + 1) * 128]
            exp_b = exppool.tile([128, N], F32, tag="exp")
            part = smpool.tile([128, NCH], F32, tag="part")
            for ck in range(NCH):
                s_ps = psum3.tile([128, CHUNK], F32, tag="sps")
                nc.tensor.matmul(s_ps[:], qt, kt_tiles[ck][:])
                nc.scalar.activation(
                    exp_b[:, ck * CHUNK : (ck + 1) * CHUNK],
                    s_ps[:],
                    AF.Exp,
                    accum_out=part[:, ck : ck + 1],
                )
            rowsum = smpool.tile([128, 1], F32, tag="rs")
            nc.vector.reduce_sum(rowsum[:], part[:], axis=AX.X)
            nc.vector.reciprocal(recip_all[:, b : b + 1], rowsum[:])
            nc.vector.tensor_scalar(
                exp_b[:], exp_b[:], recip_all[:, b : b + 1], None, op0=ALU.mult
            )
            nc.sync.dma_start(attn_w[b * 128 : (b + 1) * 128, :], exp_b[:])

        # ---------------- phase C: attn_out via key-major exp ----------------
        def phase_c(tc_i):
            pv = psum.tile([NE, CHUNK], F32, tag="pv")
            qrhs = qt_tiles[tc_i][:]
            for kt in range(NKT):
                ct_ps = psum2.tile([128, CHUNK], F32, tag="ct")
                klhs = kt_tiles[kt // 4][:, (kt % 4) * 128 : (kt % 4 + 1) * 128]
                nc.tensor.matmul(ct_ps[:], klhs, qrhs)
                expT = ctpool.tile([128, CHUNK], F32R, tag="expT")
                nc.scalar.activation(expT[:], ct_ps[:], AF.Exp)
                nc.tensor.matmul(
                    pv[:],
                    f_sb[:, kt * NE : (kt + 1) * NE],
                    expT[:],
                    start=(kt == 0),
                    stop=(kt == NKT - 1),
                )
            pv_sb = ctpool.tile([NE, CHUNK], F32, tag="pvsb")
            nc.vector.tensor_copy(pv_sb[:], pv[:])
            for i in range(CHUNK // 128):
                b = tc_i * (CHUNK // 128) + i
                tr = psum2.tile([128, NE], F32, tag="aotr")
                nc.tensor.transpose(
                    tr[:], pv_sb[:, i * 128 : (i + 1) * 128], ident[0:NE, 0:NE]
                )
                ao = smpool.tile([128, NE], F32, tag="ao")
                nc.vector.tensor_scalar(
                    ao[:], tr[:], recip_all[:, b : b + 1], None, op0=ALU.mult
                )
                nc.sync.dma_start(attn_o[b * 128 : (b + 1) * 128, :], ao[:])

        for b in range(4):
            phase_b(b)
        phase_c(0)
        for b in range(4, 8):
            phase_b(b)
        phase_c(1)

    nc.compile()
    return nc


def _prep_inputs(features, rays, scale, Wq1, bq1, Wq2, bq2, gq, bq_ln,
                 Wk1, bk1, Wk2, bk2, gk, bk_ln, Ws, bs):
    f32 = np.float32
    ns = (scale.astype(f32) @ Ws.astype(f32) + bs.astype(f32))[0]
    qscale = f32(ns) / f32(np.sqrt(f32(NE)))

    def aug(w, b):
        return np.ascontiguousarray(
            np.concatenate([w.astype(f32), b.astype(f32)[None, :]], axis=0))

    def centered(w2a, g):
        # fold LN centering + gain into the second layer: rows centered
        # over the output dim, then scaled per-output by g
        c = w2a - w2a.mean(axis=1, keepdims=True)
        return np.ascontiguousarray((c * g[None, :]).astype(f32))

    def varmat(g):
        # varB = vmat.T @ (t0*g)^2 with vmat[d',d] = 1/(64*g[d']^2)
        col = (1.0 / (NE * g.astype(np.float64) ** 2)).astype(f32)
        return np.ascontiguousarray(np.repeat(col[:, None], NE, axis=1))

    gq_s = gq.astype(f32) * qscale
    lncoef = np.ascontiguousarray(np.stack(
        [bk_ln.astype(f32), bq_ln.astype(f32) * qscale], axis=1
    ).astype(f32))

    rays2 = np.ascontiguousarray(rays.reshape(N, NPOS).astype(f32))
    common = {
        "rays_all": rays2,
        "feats": np.ascontiguousarray(features.reshape(N, NE).astype(f32)),
        "id128": np.eye(128, dtype=f32),
        "lncoef": lncoef,
        "wk1a": aug(Wk1, bk1),
        "wk2c": centered(aug(Wk2, bk2), gk.astype(f32)),
        "wq1a": aug(Wq1, bq1),
        "wq2c": centered(aug(Wq2, bq2), gq_s),
        "vark": varmat(gk.astype(f32)),
        "varq": varmat(gq_s),
    }
    in_maps = []
    for c in range(N_CORES):
        m = dict(common)
        m["rays_sh"] = np.ascontiguousarray(rays2[c * SHARD : (c + 1) * SHARD])
        in_maps.append(m)
    return in_maps


def kernel(**inputs):
    if "nc" not in _CACHE:
        _CACHE["nc"] = _build_program()
    nc = _CACHE["nc"]
    in_maps = _prep_inputs(**inputs)
    res = run_bass_kernel_spmd(nc, in_maps, core_ids=list(range(N_CORES)))
    attn_w = np.concatenate([res.results[c]["attn_w"] for c in range(N_CORES)], axis=0)
    attn_o = np.concatenate([res.results[c]["attn_o"] for c in range(N_CORES)], axis=0)
    seq, h, w = 8, 32, 32
    return attn_o.reshape(seq, h, w, NE), attn_w


# revision 30
# speedup vs baseline: 1.2539x; 1.2539x over previous
"""NeRF attention Bass kernel for 8 Trainium2 NeuronCores.

Math (from the reference):
    pos = rays.reshape(N, 16),  f = features.reshape(N, 64),  N = 8192
    Q = LN(relu(pos@Wq1+bq1)@Wq2+bq2)*gq+bq_ln / 8           [N, 64]
    K = LN(relu(pos@Wk1+bk1)@Wk2+bk2)*gk+bk_ln               [N, 64]
    attn = softmax((Q @ K.T) * norm_scale, axis=-1)          [N, N]
    attn_out = attn @ f                                      [N, 64]
    returns (attn_out.reshape(8,32,32,64), attn)

Sharding: rows of Q / attn / attn_out are split across 8 cores (1024 rows
each); K, features and the tiny MLP params are replicated.

Device-side structure (per core, all feature-major / transposed):
  1. posT [17, n] built via PE transposes of rays tiles (ones bias row).
  2. MLPs: LayerNorm's centering and gain are LINEAR, so they are folded
     into W2 host-side:  t0g = W2cg.T @ relu(W1a.T @ posT_aug) is already
     centered-and-scaled; var comes from one PE matmul with a host-built
     1/(64*g^2) broadcast matrix; then rstd (ACT sqrt + DVE reciprocal)
     and out = t0g * rstdB + b.  norm_scale/8 is folded into Q's coeffs.
  3. phase B (token-major rows): scores tile = QT_blk.T @ KT chunk (PE,
     f32r) -> exp on ACT (accum_out emits row-sums for free) -> one DVE
     tensor_scalar normalize -> one 4MB row-block DMA out.
  4. phase C (key-major): scoresT = KT_tile.T @ QT (PE) -> exp (ACT) ->
     PV matmul accumulating attn_out.T over 64 key tiles; PE-transpose
     back to token-major and scale by the phase-B reciprocals.

Softmax skips the max-subtraction: scores are bounded (|s| < ~6 for this
problem family) so exp never overflows; this matches jax to ~1e-5.
Matmuls read fp32 data as float32r (fp22) which runs the PE at full rate.
"""

from contextlib import ExitStack

import numpy as np

import concourse.bass as bass
import concourse.mybir as mybir
import concourse.tile as tile
from concourse import bacc
from concourse.bass_utils import run_bass_kernel_spmd

F32 = mybir.dt.float32
F32R = mybir.dt.float32r
AF = mybir.ActivationFunctionType
ALU = mybir.AluOpType
AX = mybir.AxisListType

N_CORES = 8
N = 8192          # total tokens (8*32*32)
NPOS = 16
NHID = 32
NE = 64           # embed dim == feature dim
SHARD = N // N_CORES          # 1024 rows per core
CHUNK = 512                   # free-dim chunk for matmuls / MLP
NCH = N // CHUNK              # 16 K-chunks
NCH_Q = SHARD // CHUNK        # 2 Q-chunks
NBLK = SHARD // 128           # 8 token blocks per core
NKT = N // 128                # 64 key tiles
EPS = 1e-5

_CACHE = {}


def _r(ap):
    """Read an fp32 AP as float32r (fp22 truncated, full-rate PE)."""
    return ap.bitcast(F32R)


def _build_program():
    nc = bacc.Bacc("TRN2", target_bir_lowering=False, debug=False)

    rays_all = nc.dram_tensor("rays_all", [N, NPOS], F32, kind="ExternalInput").ap()
    rays_sh = nc.dram_tensor("rays_sh", [SHARD, NPOS], F32, kind="ExternalInput").ap()
    feats = nc.dram_tensor("feats", [N, NE], F32R, kind="ExternalInput").ap()
    id128 = nc.dram_tensor("id128", [128, 128], F32, kind="ExternalInput").ap()
    # per-feature LN shift columns: bk_ln, bq_ln' (q pre-scaled by ns/8)
    lncoef = nc.dram_tensor("lncoef", [NE, 2], F32, kind="ExternalInput").ap()
    wk1a = nc.dram_tensor("wk1a", [NPOS + 1, NHID], F32R, kind="ExternalInput").ap()
    wk2c = nc.dram_tensor("wk2c", [NHID + 1, NE], F32R, kind="ExternalInput").ap()
    wq1a = nc.dram_tensor("wq1a", [NPOS + 1, NHID], F32R, kind="ExternalInput").ap()
    wq2c = nc.dram_tensor("wq2c", [NHID + 1, NE], F32R, kind="ExternalInput").ap()
    vark = nc.dram_tensor("vark", [NE, NE], F32R, kind="ExternalInput").ap()
    varq = nc.dram_tensor("varq", [NE, NE], F32R, kind="ExternalInput").ap()

    attn_w = nc.dram_tensor("attn_w", [SHARD, N], F32, kind="ExternalOutput").ap()
    attn_o = nc.dram_tensor("attn_o", [SHARD, NE], F32, kind="ExternalOutput").ap()

    with tile.TileContext(nc) as tc, ExitStack() as ctx:
        const = ctx.enter_context(tc.tile_pool(name="const", bufs=1))
        persist = ctx.enter_context(tc.tile_pool(name="persist", bufs=1))

        # ---------------- constants / inputs to SBUF ----------------
        ident = const.tile([128, 128], F32, tag="ident")
        nc.sync.dma_start(ident[:], id128)
        lnc = const.tile([NE, 2], F32, tag="lnc")
        nc.sync.dma_start(lnc[:], lncoef)
        w_k1 = const.tile([NPOS + 1, NHID], F32R, tag="wk1")
        nc.sync.dma_start(w_k1[:], wk1a)
        w_k2 = const.tile([NHID + 1, NE], F32R, tag="wk2")
        nc.sync.dma_start(w_k2[:], wk2c)
        w_q1 = const.tile([NPOS + 1, NHID], F32R, tag="wq1")
        nc.sync.dma_start(w_q1[:], wq1a)
        w_q2 = const.tile([NHID + 1, NE], F32R, tag="wq2")
        nc.sync.dma_start(w_q2[:], wq2c)
        v_k = const.tile([NE, NE], F32R, tag="vark")
        nc.sync.dma_start(v_k[:], vark)
        v_q = const.tile([NE, NE], F32R, tag="varq")
        nc.sync.dma_start(v_q[:], varq)
        eps_col = const.tile([128, 1], F32, tag="eps")
        nc.vector.memset(eps_col[:], EPS)

        # features, token-tiled: f_sb[:, kt*64:(kt+1)*64] = feats[kt*128:+128, :]
        f_sb = persist.tile([128, NKT * NE], F32R, tag="f")
        nc.sync.dma_start(
            f_sb[:].rearrange("p (t d) -> p t d", t=NKT),
            feats.rearrange("(t p) d -> p t d", p=128),
        )

        # ---------------- posT via PE transposes ----------------
        def build_posT(rays_ap, n_tok, tag, mlppool, psA):
            nt = n_tok // 128
            pos_sb = mlppool.tile([128, nt * NPOS], F32, tag=tag + "_tm")
            nc.sync.dma_start(
                pos_sb[:].rearrange("p (t j) -> p t j", t=nt),
                rays_ap.rearrange("(t p) j -> p t j", p=128),
            )
            posTa = mlppool.tile([NPOS + 1, n_tok], F32R, tag=tag)
            # bias row (NPOS) must be ones; memset whole tile, transposes
            # overwrite rows 0-15 (DVE needs 32-aligned start partitions)
            nc.vector.memset(posTa[:].bitcast(F32), 1.0)
            for g in range(0, nt, 4):
                tr = psA.tile([NPOS, 4 * 128], F32, tag="ptr")
                for i in range(4):
                    t = g + i
                    nc.tensor.transpose(
                        tr[:, i * 128 : (i + 1) * 128],
                        pos_sb[:, t * NPOS : (t + 1) * NPOS],
                        ident[:],
                    )
                nc.vector.tensor_copy(posTa[0:NPOS, g * 128 : (g + 4) * 128], tr[:])
            return posTa

        # ---------------- feature-major MLP -> transposed embeddings ----------
        def mlp(posTa, w1, w2c, vmat, b_col, nch, out_aps, mlpsb, psA, psB):
            """Returns nch tiles [NE, CHUNK]: the normalized embedding^T.

            w2c is the centered-and-gain-scaled second layer, so its matmul
            output t0g is (a - mean(a)) * g directly; vmat = 1/(64*g^2)
            broadcast matrix gives varB = var(a) replicated on every row.
            """
            out_tiles = []
            for c in range(nch):
                cols = slice(c * CHUNK, (c + 1) * CHUNK)
                h_ps = psA.tile([NHID, CHUNK], F32, tag="mlp_h")
                nc.tensor.matmul(h_ps[:], w1[:], _r(posTa[:, cols]))
                h_sb = mlpsb.tile([NHID + 1, CHUNK], F32R, tag="mlp_h")
                nc.vector.memset(h_sb[NHID : NHID + 1, :].bitcast(F32), 1.0)
                nc.vector.tensor_scalar(
                    h_sb[0:NHID, :], h_ps[:], 0.0, None, op0=ALU.max
                )
                t0_ps = psA.tile([NE, CHUNK], F32, tag="mlp_t0")
                nc.tensor.matmul(t0_ps[:], w2c[:], h_sb[:])
                t0_sb = mlpsb.tile([NE, CHUNK], F32R, tag="mlp_t0")
                nc.vector.tensor_copy(t0_sb[:], t0_ps[:])
                sqd = mlpsb.tile([NE, CHUNK], F32R, tag="mlp_sqd")
                nc.vector.tensor_tensor(
                    sqd[:], t0_sb[:].bitcast(F32), t0_sb[:].bitcast(F32), op=ALU.mult
                )
                varB = psB.tile([NE, CHUNK], F32, tag="varB")
                nc.tensor.matmul(varB[:], vmat[:], sqd[:])
                sqB = mlpsb.tile([NE, CHUNK], F32, tag="mlp_sqB")
                nc.scalar.activation(sqB[:], varB[:], AF.Sqrt, bias=eps_col[0:NE, :])
                rstdB = mlpsb.tile([NE, CHUNK], F32, tag="mlp_rstd")
                nc.vector.reciprocal_approx_fast(rstdB[:], sqB[:])
                t1 = mlpsb.tile([NE, CHUNK], F32, tag="mlp_t1")
                nc.vector.tensor_tensor(
                    t1[:], t0_sb[:].bitcast(F32), rstdB[:], op=ALU.mult
                )
                o = out_aps[c]
                nc.vector.tensor_scalar(o, t1[:], b_col, None, op0=ALU.add)
                out_tiles.append(o)
            return out_tiles

        with (
            tc.tile_pool(name="mlppool", bufs=1) as mlppool,
            tc.tile_pool(name="mlpsb", bufs=3) as mlpsb,
            tc.tile_pool(name="psA", bufs=2, space="PSUM") as psA,
            tc.tile_pool(name="psB", bufs=2, space="PSUM") as psB,
        ):
            posT_k = build_posT(rays_all, N, "posk", mlppool, psA)
            posT_q = build_posT(rays_sh, SHARD, "posq", mlppool, psA)

            # paired layout: kt_pair[p] rows 0-63 = K^T chunk 2p, rows 64-127
            # = chunk 2p+1 (key tiles 8p+4..8p+7); qt_pair[t] duplicates Q^T
            # chunk t in both halves.  Lets scores matmuls run as row-packed
            # pairs (tile_position (0,0) + (64,0)) using both array halves.
            kt_pair = [persist.tile([128, CHUNK], F32R, tag=f"ktp{p}",
                                    name=f"ktp{p}")
                       for p in range(NCH // 2)]
            qt_pair = [persist.tile([128, CHUNK], F32R, tag=f"qtp{t}",
                                    name=f"qtp{t}")
                       for t in range(NCH_Q)]
            k_out, k_tmp = [], []
            for c in range(NCH):
                if c % 2 == 0:
                    k_out.append(kt_pair[c // 2][0:64, :])
                else:
                    t = mlppool.tile([NE, CHUNK], F32R, tag=f"ktmp{c}",
                                       name=f"ktmp{c}")
                    k_tmp.append(t)
                    k_out.append(t[:])
            q_out = []
            for c in range(NCH_Q):
                q_out.append(qt_pair[c][0:64, :])
            mlp(posT_k, w_k1, w_k2, v_k, lnc[:, 0:1], NCH, k_out,
                mlpsb, psA, psB)
            mlp(posT_q, w_q1, w_q2, v_q, lnc[:, 1:2], NCH_Q, q_out,
                mlpsb, psA, psB)
            # shift odd K chunks / Q dup into partitions 64-127 via DMA
            for c in range(1, NCH, 2):
                nc.sync.dma_start(kt_pair[c // 2][64:128, :],
                                  k_tmp[c // 2][:])
            for t in range(NCH_Q):
                nc.sync.dma_start(qt_pair[t][64:128, :], qt_pair[t][0:64, :])

        exppool = ctx.enter_context(tc.tile_pool(name="exppool", bufs=2))
        ctpool = ctx.enter_context(tc.tile_pool(name="ctpool", bufs=4))
        smpool = ctx.enter_context(tc.tile_pool(name="smpool", bufs=3))

        recip_all = persist.tile([128, NBLK], F32, tag="recip")

        # ---------------- phase B: token-major attn rows ----------------
        # scores arrive in [128, 2048] 4-bank PSUM groups (4 K-chunks via two
        # row-packed matmul pairs), one batched exp per group.
        with tc.tile_pool(name="psumB", bufs=2, space="PSUM") as psumB:
            for b in range(NBLK):
                tcq = b // 4
                bcols = slice((b % 4) * 128, (b % 4 + 1) * 128)
                exp_b = exppool.tile([128, N], F32, tag="exp")
                part = smpool.tile([128, 4], F32, tag="part")
                for g in range(4):
                    s_ps = psumB.tile([128, 4 * CHUNK], F32, tag="sps")
                    for j in range(2):
                        p = g * 2 + j
                        nc.tensor.matmul(
                            s_ps[:, (2 * j) * CHUNK : (2 * j + 1) * CHUNK],
                            qt_pair[tcq][0:64, bcols], kt_pair[p][0:64, :],
                        )
                        nc.tensor.matmul(
                            s_ps[:, (2 * j + 1) * CHUNK : (2 * j + 2) * CHUNK],
                            qt_pair[tcq][64:128, bcols], kt_pair[p][64:128, :],
                        )
                    nc.scalar.activation(
                        exp_b[:, g * 4 * CHUNK : (g + 1) * 4 * CHUNK],
                        s_ps[:],
                        AF.Exp,
                        accum_out=part[:, g : g + 1],
                    )
                rowsum = smpool.tile([128, 1], F32, tag="rs")
                nc.vector.reduce_sum(rowsum[:], part[:], axis=AX.X)
                nc.vector.reciprocal(recip_all[:, b : b + 1], rowsum[:])
                nc.vector.tensor_scalar(
                    exp_b[:], exp_b[:], recip_all[:, b : b + 1], None, op0=ALU.mult
                )
                nc.sync.dma_start(attn_w[b * 128 : (b + 1) * 128, :], exp_b[:])

        # ---------------- phase C: attn_out via key-major exp ----------------
        with (
            tc.tile_pool(name="psumC", bufs=3, space="PSUM") as psumC,
            tc.tile_pool(name="psumP", bufs=1, space="PSUM") as psumP,
            tc.tile_pool(name="psumT", bufs=1, space="PSUM") as psumT,
        ):
            for tc_i in range(NCH_Q):
                pv = psumP.tile([NE, CHUNK], F32, tag="pv")
                for gi in range(NKT // 2):
                    pp, jj = gi // 4, gi % 4
                    ka = pp * 8 + jj
                    kb = pp * 8 + 4 + jj
                    ct = psumC.tile([128, 2 * CHUNK], F32, tag="ct")
                    nc.tensor.matmul(
                        ct[:, 0:CHUNK],
                        kt_pair[pp][0:64, jj * 128 : (jj + 1) * 128],
                        qt_pair[tc_i][0:64, :],
                    )
                    nc.tensor.matmul(
                        ct[:, CHUNK : 2 * CHUNK],
                        kt_pair[pp][64:128, jj * 128 : (jj + 1) * 128],
                        qt_pair[tc_i][64:128, :],
                    )
                    expT = ctpool.tile([128, 2 * CHUNK], F32R, tag="expT")
                    nc.scalar.activation(expT[:], ct[:], AF.Exp)
                    nc.tensor.matmul(
                        pv[:], f_sb[:, ka * NE : (ka + 1) * NE],
                        expT[:, 0:CHUNK],
                        start=(gi == 0), stop=False,
                    )
                    nc.tensor.matmul(
                        pv[:], f_sb[:, kb * NE : (kb + 1) * NE],
                        expT[:, CHUNK : 2 * CHUNK],
                        start=False, stop=(gi == NKT // 2 - 1),
                    )
                pv_sb = ctpool.tile([NE, CHUNK], F32, tag="pvsb")
                nc.vector.tensor_copy(pv_sb[:], pv[:])
                for i in range(CHUNK // 128):
                    b = tc_i * (CHUNK // 128) + i
                    tr = psumT.tile([128, NE], F32, tag="aotr")
                    nc.tensor.transpose(
                        tr[:], pv_sb[:, i * 128 : (i + 1) * 128],
                        ident[0:NE, 0:NE],
                    )
                    ao = smpool.tile([128, NE], F32, tag="ao")
                    nc.vector.tensor_scalar(
                        ao[:], tr[:], recip_all[:, b : b + 1], None, op0=ALU.mult
                    )
                    nc.sync.dma_start(attn_o[b * 128 : (b + 1) * 128, :], ao[:])

    nc.compile()
    return nc


def _prep_inputs(features, rays, scale, Wq1, bq1, Wq2, bq2, gq, bq_ln,
                 Wk1, bk1, Wk2, bk2, gk, bk_ln, Ws, bs):
    f32 = np.float32
    ns = (scale.astype(f32) @ Ws.astype(f32) + bs.astype(f32))[0]
    qscale = f32(ns) / f32(np.sqrt(f32(NE)))

    def aug(w, b):
        return np.ascontiguousarray(
            np.concatenate([w.astype(f32), b.astype(f32)[None, :]], axis=0))

    def centered(w2a, g):
        # fold LN centering + gain into the second layer: rows centered
        # over the output dim, then scaled per-output by g
        c = w2a - w2a.mean(axis=1, keepdims=True)
        return np.ascontiguousarray((c * g[None, :]).astype(f32))

    def varmat(g):
        # varB = vmat.T @ (t0*g)^2 with vmat[d',d] = 1/(64*g[d']^2)
        col = (1.0 / (NE * g.astype(np.float64) ** 2)).astype(f32)
        return np.ascontiguousarray(np.repeat(col[:, None], NE, axis=1))

    gq_s = gq.astype(f32) * qscale
    lncoef = np.ascontiguousarray(np.stack(
        [bk_ln.astype(f32), bq_ln.astype(f32) * qscale], axis=1
    ).astype(f32))

    rays2 = np.ascontiguousarray(rays.reshape(N, NPOS).astype(f32))
    common = {
        "rays_all": rays2,
        "feats": np.ascontiguousarray(features.reshape(N, NE).astype(f32)),
        "id128": np.eye(128, dtype=f32),
        "lncoef": lncoef,
        "wk1a": aug(Wk1, bk1),
        "wk2c": centered(aug(Wk2, bk2), gk.astype(f32)),
        "wq1a": aug(Wq1, bq1),
        "wq2c": centered(aug(Wq2, bq2), gq_s),
        "vark": varmat(gk.astype(f32)),
        "varq": varmat(gq_s),
    }
    in_maps = []
    for c in range(N_CORES):
        m = dict(common)
        m["rays_sh"] = np.ascontiguousarray(rays2[c * SHARD : (c + 1) * SHARD])
        in_maps.append(m)
    return in_maps


def kernel(**inputs):
    if "nc" not in _CACHE:
        _CACHE["nc"] = _build_program()
    nc = _CACHE["nc"]
    in_maps = _prep_inputs(**inputs)
    res = run_bass_kernel_spmd(nc, in_maps, core_ids=list(range(N_CORES)))
    attn_w = np.concatenate([res.results[c]["attn_w"] for c in range(N_CORES)], axis=0)
    attn_o = np.concatenate([res.results[c]["attn_o"] for c in range(N_CORES)], axis=0)
    seq, h, w = 8, 32, 32
    return attn_o.reshape(seq, h, w, NE), attn_w
